# revision 2
# baseline (speedup 1.0000x reference)
"""Batch-parallel attention kernel for Trainium2 (8 NeuronCores).

Problem: out[b,j,d] = sum_i softmax_j(enc[b] @ dec[b].T)[i,j] * enc[b,i,d]
  enc/dec: [8, 2048, 512] fp32.  One batch per core (data parallel).

Per-core algorithm (batch b):
  S = enc @ dec.T        [2048, 2048]  (fp16 hi/lo 3-pass matmul, fp32-accurate)
  A = softmax(S, axis=1) (row max via DVE reduce, exp via ACT with bias+accum)
  out = A.T @ enc        (fp16 matmul; 1/L folded into enc rows)

Matmul layouts (out = lhsT.T @ rhs, contraction over partitions):
  MM1: lhsT = encT [d,i] chunks, rhs = decT [d,j] -> S[i,j] in PSUM.
       encT/decT produced by PE-transpose (identity matmul), split into
       fp16 hi + fp16 lo planes for the 3-pass product.
  MM2: lhsT = P[i,j] block (natural layout), rhs = enc_scaled[i,d] fp16.
"""

import os
import sys

sys.path.insert(0, "/opt/trn_rl_repo")

from contextlib import ExitStack

import numpy as np

import concourse.bass as bass  # noqa: F401  (engine types referenced via nc)
import concourse.bacc as bacc
import concourse.mybir as mybir
import concourse.tile as tile
from concourse.masks import make_identity
from concourse.bass_utils import run_bass_kernel_spmd

F32 = mybir.dt.float32
F16 = mybir.dt.float16
AX = mybir.AxisListType
ALU = mybir.AluOpType
ACTF = mybir.ActivationFunctionType

B, S_LEN, D = 8, 2048, 512
IB = S_LEN // 128   # 16 row blocks
KC = D // 128       # 4 contraction chunks
NT = S_LEN // 512   # 4 wide j tiles
JT = S_LEN // 128   # 16 out row blocks

LAST_EXEC_NS = None


def _build():
    nc = bacc.Bacc()
    enc = nc.declare_dram_parameter("enc", [S_LEN, D], F32, isOutput=False)
    dec = nc.declare_dram_parameter("dec", [S_LEN, D], F32, isOutput=False)
    out = nc.declare_dram_parameter("out", [S_LEN, D], F32, isOutput=True)

    with ExitStack() as ctx:
        tc = ctx.enter_context(tile.TileContext(nc))
        singles = ctx.enter_context(tc.tile_pool(name="singles", bufs=1))
        ld = ctx.enter_context(tc.tile_pool(name="ld", bufs=3))
        small = ctx.enter_context(tc.tile_pool(name="small", bufs=8))
        stage = ctx.enter_context(tc.tile_pool(name="stage", bufs=3))

        ident = singles.tile([128, 128], F32)
        make_identity(nc, ident)

        # resident tensors
        encT_hi = singles.tile([128, KC, S_LEN], F16)   # [d, kc, i]
        encT_lo = singles.tile([128, KC, S_LEN], F16)
        decT_hi = singles.tile([128, KC, S_LEN], F16)   # [d, kc, j]
        decT_lo = singles.tile([128, KC, S_LEN], F16)
        enc_res = singles.tile([128, IB, D], F32)       # [i, ib, d]
        enc_sc = singles.tile([128, IB, D], F16)        # [i, ib, d] scaled by 1/L
        P = singles.tile([128, IB, S_LEN], F16)         # [i, ib, j]

        # ---------------- phase 0: load + transpose + hi/lo split ----------
        with tc.tile_pool(name="psum_t", bufs=4, space="PSUM") as psum_t:
            for ib in range(IB):
                nc.sync.dma_start(out=enc_res[:, ib, :], in_=enc[ib * 128:(ib + 1) * 128, :])

            def transp(src_ap, dst_hi, dst_lo, jsl):
                pt = psum_t.tile([128, 128], F32, tag="pt", name="pt")
                nc.tensor.transpose(pt, src_ap, ident)
                nc.scalar.copy(out=dst_hi[:, jsl], in_=pt)
                nc.vector.scalar_tensor_tensor(
                    out=dst_lo[:, jsl], in0=dst_hi[:, jsl], scalar=-1.0,
                    in1=pt, op0=ALU.mult, op1=ALU.add)

            for jb in range(IB):
                dec_sb = ld.tile([128, D], F32, tag="dec_sb", name="dec_sb")
                nc.sync.dma_start(out=dec_sb, in_=dec[jb * 128:(jb + 1) * 128, :])
                for k in range(KC):
                    transp(dec_sb[:, k * 128:(k + 1) * 128],
                           decT_hi[:, k], decT_lo[:, k],
                           slice(jb * 128, (jb + 1) * 128))
            for ib in range(IB):
                for k in range(KC):
                    transp(enc_res[:, ib, k * 128:(k + 1) * 128],
                           encT_hi[:, k], encT_lo[:, k],
                           slice(ib * 128, (ib + 1) * 128))

        # ---------------- phase 1: S = enc@dec.T, softmax, P --------------
        with tc.tile_pool(name="psum_s", bufs=2, space="PSUM") as psum_s:
            for ib in range(IB):
                isl = slice(ib * 128, (ib + 1) * 128)
                S = psum_s.tile([128, S_LEN], F32, tag="S", name="S")
                passes = [(encT_hi, decT_hi), (encT_hi, decT_lo), (encT_lo, decT_hi)]
                for pi, (eT, dT) in enumerate(passes):
                    for k in range(KC):
                        for n in range(NT):
                            nc.tensor.matmul(
                                S[:, n * 512:(n + 1) * 512],
                                lhsT=eT[:, k, isl],
                                rhs=dT[:, k, n * 512:(n + 1) * 512],
                                start=(pi == 0 and k == 0),
                                stop=(pi == 2 and k == KC - 1))

                nm = [small.tile([128, 1], F32, tag=f"nm{i}", name=f"nm{i}")
                      for i in range(NT)]
                for n in range(NT):
                    nc.vector.tensor_reduce(out=nm[n], in_=S[:, n * 512:(n + 1) * 512],
                                            axis=AX.X, op=ALU.max, negate=True)
                nc.vector.tensor_tensor(out=nm[0], in0=nm[0], in1=nm[1], op=ALU.min)
                nc.vector.tensor_tensor(out=nm[2], in0=nm[2], in1=nm[3], op=ALU.min)
                negm = small.tile([128, 1], F32, tag="negm", name="negm")
                nc.vector.tensor_tensor(out=negm, in0=nm[0], in1=nm[2], op=ALU.min)

                lpart = small.tile([128, NT], F32, tag="lpart", name="lpart")
                for n in range(NT):
                    nc.scalar.activation(out=P[:, ib, n * 512:(n + 1) * 512],
                                         in_=S[:, n * 512:(n + 1) * 512],
                                         func=ACTF.Exp, bias=negm, scale=1.0,
                                         accum_out=lpart[:, n:n + 1])
                lsum = small.tile([128, 1], F32, tag="lsum", name="lsum")
                nc.vector.tensor_reduce(out=lsum, in_=lpart, axis=AX.X, op=ALU.add)
                rec = small.tile([128, 1], F32, tag="rec", name="rec")
                nc.vector.reciprocal(out=rec, in_=lsum)
                nc.vector.tensor_scalar(out=enc_sc[:, ib, :], in0=enc_res[:, ib, :],
                                        scalar1=rec, scalar2=None, op0=ALU.mult)

        # ---------------- phase 2: out = A.T @ enc_scaled ------------------
        with tc.tile_pool(name="psum_o", bufs=4, space="PSUM") as psum_o:
            for jt in range(JT):
                po = psum_o.tile([128, D], F32, tag="po", name="po")
                for ib in range(IB):
                    nc.tensor.matmul(po,
                                     lhsT=P[:, ib, jt * 128:(jt + 1) * 128],
                                     rhs=enc_sc[:, ib, :],
                                     start=(ib == 0), stop=(ib == IB - 1))
                st = stage.tile([128, D], F32, tag="st", name="st")
                nc.scalar.copy(out=st, in_=po)
                nc.sync.dma_start(out=out[jt * 128:(jt + 1) * 128, :], in_=st)

    nc.compile()
    return nc


_NC = None


def kernel(enc_outputs, dec_outputs):
    global _NC, LAST_EXEC_NS
    enc_outputs = np.ascontiguousarray(np.asarray(enc_outputs, dtype=np.float32))
    dec_outputs = np.ascontiguousarray(np.asarray(dec_outputs, dtype=np.float32))
    assert enc_outputs.shape == (B, S_LEN, D), enc_outputs.shape
    assert dec_outputs.shape == (B, S_LEN, D), dec_outputs.shape

    if _NC is None:
        _NC = _build()

    in_maps = [{"enc": enc_outputs[b], "dec": dec_outputs[b]} for b in range(B)]
    trace = bool(int(os.environ.get("BASS_ATTN_TRACE", "0")))
    if trace:
        try:
            res = run_bass_kernel_spmd(_NC, in_maps, core_ids=list(range(B)), trace=True)
        except Exception:
            res = run_bass_kernel_spmd(_NC, in_maps, core_ids=list(range(B)))
    else:
        res = run_bass_kernel_spmd(_NC, in_maps, core_ids=list(range(B)))
    LAST_EXEC_NS = res.exec_time_ns
    return np.stack([res.results[b]["out"] for b in range(B)], axis=0)


# revision 6
# speedup vs baseline: 1.1563x; 1.1563x over previous
"""Batch-parallel attention kernel for Trainium2 (8 NeuronCores).

Problem: out[b,j,d] = sum_i softmax_j(enc[b] @ dec[b].T)[i,j] * enc[b,i,d]
  enc/dec: [8, 2048, 512] fp32.  One batch per core (data parallel).

Per-core algorithm (batch b):
  S = enc @ dec.T        [2048, 2048]  (fp16 hi/lo 3-pass matmul, fp32-grade)
  A = softmax(S, axis=1) (two-half online softmax: local max/sum per
                          1024-wide half, combined into global stats)
  out = A.T @ enc        (fp16 matmul; exp(m_h-m)/L folded into enc rows,
                          one scaled enc variant per half)

Matmul layouts (out = lhsT.T @ rhs, contraction over partitions):
  MM1: lhsT = encT [d,i] chunks, rhs = decT [d,j] -> S[i,j] in PSUM.
       encT/decT produced by PE-transpose (identity matmul), split into
       fp16 hi + fp16 lo planes for the 3-pass product.
  MM2: lhsT = P[i,j] block (natural layout), rhs = enc_scaled[i,h,d] fp16.
"""

import os
import sys

sys.path.insert(0, "/opt/trn_rl_repo")

from contextlib import ExitStack

import numpy as np

import concourse.bass as bass  # noqa: F401
import concourse.bacc as bacc
import concourse.mybir as mybir
import concourse.tile as tile
from concourse.masks import make_identity
from concourse.bass_utils import run_bass_kernel_spmd

F32 = mybir.dt.float32
F16 = mybir.dt.float16
AX = mybir.AxisListType
ALU = mybir.AluOpType
ACTF = mybir.ActivationFunctionType

B, S_LEN, D = 8, 2048, 512
IB = S_LEN // 128   # 16 row blocks
KC = D // 128       # 4 contraction chunks
JT = S_LEN // 128   # 16 out row blocks
HALF = S_LEN // 2   # 1024

LAST_EXEC_NS = None


def _build(repeat=1, mm1_passes=3, skip_mm2=False):
    nc = bacc.Bacc()
    enc = nc.declare_dram_parameter("enc", [S_LEN, D], F32, isOutput=False)
    dec = nc.declare_dram_parameter("dec", [S_LEN, D], F32, isOutput=False)
    out = nc.declare_dram_parameter("out", [S_LEN, D], F32, isOutput=True)

    with ExitStack() as ctx:
        tc = ctx.enter_context(tile.TileContext(nc))
        if repeat > 1:
            ctx.enter_context(tc.For_i(0, repeat, 1))
        singles = ctx.enter_context(tc.tile_pool(name="singles", bufs=1))
        ld = ctx.enter_context(tc.tile_pool(name="ld", bufs=3))
        small = ctx.enter_context(tc.tile_pool(name="small", bufs=4))
        stage = ctx.enter_context(tc.tile_pool(name="stage", bufs=3))
        psum_t = ctx.enter_context(tc.tile_pool(name="psum_t", bufs=2, space="PSUM"))
        psum_s = ctx.enter_context(tc.tile_pool(name="psum_s", bufs=2, space="PSUM"))
        psum_o = ctx.enter_context(tc.tile_pool(name="psum_o", bufs=2, space="PSUM"))

        ident = singles.tile([128, 128], F32)
        make_identity(nc, ident)

        # resident tensors (per-partition bytes: 4*16K f16 = 64K + P 64K + 16K)
        encT_hi = singles.tile([128, KC, S_LEN], F16)   # [d, kc, i]
        encT_lo = singles.tile([128, KC, S_LEN], F16)
        decT_hi = singles.tile([128, KC, S_LEN], F16)   # [d, kc, j]
        decT_lo = singles.tile([128, KC, S_LEN], F16)
        enc_sc = singles.tile([128, IB, 2, D], F16)     # [i, ib, half, d]
        P = singles.tile([128, IB, S_LEN], F16)         # [i, ib, j]

        def transp4(src_sb, dst_hi, dst_lo, jsl):
            # transpose 4 [128,128] chunks of src_sb [128,512] into one PSUM
            # bank, then split to fp16 hi/lo planes with one wide op each
            pt = psum_t.tile([128, 512], F32, tag="pt", name="pt")
            for k in range(KC):
                nc.tensor.transpose(pt[:, k * 128:(k + 1) * 128],
                                    src_sb[:, k * 128:(k + 1) * 128], ident)
            ptv = pt.rearrange("p (k c) -> p k c", k=KC)
            nc.scalar.copy(out=dst_hi[:, :, jsl], in_=ptv)
            nc.vector.scalar_tensor_tensor(
                out=dst_lo[:, :, jsl], in0=dst_hi[:, :, jsl], scalar=-1.0,
                in1=ptv, op0=ALU.mult, op1=ALU.add)

        # ---- dec: load + transpose + split (before MM1 needs decT) --------
        for jb in range(IB):
            dec_sb = ld.tile([128, D], F32, tag="dec_sb", name="dec_sb")
            nc.sync.dma_start(out=dec_sb, in_=dec[jb * 128:(jb + 1) * 128, :])
            transp4(dec_sb, decT_hi, decT_lo, slice(jb * 128, (jb + 1) * 128))

        # ---- phase 1: per row block: enc transp, S halves, online softmax -
        # enc transposes run 2 iterations ahead so their ACT/DVE hi/lo splits
        # complete off the PE critical path; enc_sb tiles stay live until the
        # end-of-iteration scaling, so give them their own deep pool
        enc_ld = ctx.enter_context(tc.tile_pool(name="enc_ld", bufs=4))
        enc_sbs = {}

        def prefetch_enc(ib):
            if ib >= IB:
                return
            enc_sb = enc_ld.tile([128, D], F32, tag="enc_sb", name="enc_sb")
            nc.sync.dma_start(out=enc_sb, in_=enc[ib * 128:(ib + 1) * 128, :])
            transp4(enc_sb, encT_hi, encT_lo, slice(ib * 128, (ib + 1) * 128))
            enc_sbs[ib] = enc_sb

        prefetch_enc(0)
        prefetch_enc(1)
        for ib in range(IB):
            isl = slice(ib * 128, (ib + 1) * 128)
            prefetch_enc(ib + 2)
            enc_sb = enc_sbs.pop(ib)

            passes = [(encT_hi, decT_hi), (encT_hi, decT_lo),
                      (encT_lo, decT_hi)][:mm1_passes]
            npass = len(passes)
            nm = [None, None]
            lp = [None, None]
            for h in range(2):
                Sh = psum_s.tile([128, HALF], F32, tag="S", name="S")
                for pi, (eT, dT) in enumerate(passes):
                    for k in range(KC):
                        for n in range(2):
                            nc.tensor.matmul(
                                Sh[:, n * 512:(n + 1) * 512],
                                lhsT=eT[:, k, isl],
                                rhs=dT[:, k, h * HALF + n * 512: h * HALF + (n + 1) * 512],
                                start=(pi == 0 and k == 0),
                                stop=(pi == npass - 1 and k == KC - 1))
                nm[h] = small.tile([128, 1], F32, tag=f"nm{h}", name=f"nm{h}")
                nc.vector.tensor_reduce(out=nm[h], in_=Sh, axis=AX.X,
                                        op=ALU.max, negate=True)
                lp[h] = small.tile([128, 1], F32, tag=f"lp{h}", name=f"lp{h}")
                nc.scalar.activation(out=P[:, ib, h * HALF:(h + 1) * HALF],
                                     in_=Sh, func=ACTF.Exp, bias=nm[h],
                                     scale=1.0, accum_out=lp[h])

            # combine halves: m = max(m0, m1); f_h = exp(m_h - m);
            # L = L0 f0 + L1 f1; r_h = f_h / L; enc_sc_h = enc * r_h
            negm = small.tile([128, 1], F32, tag="negm", name="negm")
            nc.vector.tensor_tensor(out=negm, in0=nm[0], in1=nm[1], op=ALU.min)
            f = [None, None]
            for h in range(2):
                df = small.tile([128, 1], F32, tag=f"df{h}", name=f"df{h}")
                nc.vector.tensor_tensor(out=df, in0=negm, in1=nm[h], op=ALU.subtract)
                f[h] = small.tile([128, 1], F32, tag=f"f{h}", name=f"f{h}")
                nc.scalar.activation(out=f[h], in_=df, func=ACTF.Exp)
            l1f1 = small.tile([128, 1], F32, tag="l1f1", name="l1f1")
            nc.vector.tensor_tensor(out=l1f1, in0=lp[1], in1=f[1], op=ALU.mult)
            L = small.tile([128, 1], F32, tag="L", name="L")
            nc.vector.scalar_tensor_tensor(out=L, in0=lp[0], scalar=f[0],
                                           in1=l1f1, op0=ALU.mult, op1=ALU.add)
            r = small.tile([128, 1], F32, tag="r", name="r")
            nc.vector.reciprocal(out=r, in_=L)
            for h in range(2):
                rh = small.tile([128, 1], F32, tag=f"rh{h}", name=f"rh{h}")
                nc.vector.tensor_tensor(out=rh, in0=r, in1=f[h], op=ALU.mult)
                nc.vector.tensor_scalar(out=enc_sc[:, ib, h, :], in0=enc_sb,
                                        scalar1=rh, scalar2=None, op0=ALU.mult)

        # ---- phase 2: out = A.T @ enc_scaled ------------------------------
        for jt in range(JT if not skip_mm2 else 1):
            po = psum_o.tile([128, D], F32, tag="po", name="po")
            h = jt // (JT // 2)
            for ib in range(IB):
                nc.tensor.matmul(po,
                                 lhsT=P[:, ib, jt * 128:(jt + 1) * 128],
                                 rhs=enc_sc[:, ib, h, :],
                                 start=(ib == 0), stop=(ib == IB - 1))
            st = stage.tile([128, D], F32, tag="st", name="st")
            nc.scalar.copy(out=st, in_=po)
            nc.sync.dma_start(out=out[jt * 128:(jt + 1) * 128, :], in_=st)

    nc.compile()
    return nc


_NC = None


def kernel(enc_outputs, dec_outputs):
    global _NC, LAST_EXEC_NS
    enc_outputs = np.ascontiguousarray(np.asarray(enc_outputs, dtype=np.float32))
    dec_outputs = np.ascontiguousarray(np.asarray(dec_outputs, dtype=np.float32))
    assert enc_outputs.shape == (B, S_LEN, D), enc_outputs.shape
    assert dec_outputs.shape == (B, S_LEN, D), dec_outputs.shape

    if _NC is None:
        _NC = _build()

    in_maps = [{"enc": enc_outputs[b], "dec": dec_outputs[b]} for b in range(B)]
    trace = bool(int(os.environ.get("BASS_ATTN_TRACE", "0")))
    if trace:
        try:
            res = run_bass_kernel_spmd(_NC, in_maps, core_ids=list(range(B)), trace=True)
        except Exception:
            res = run_bass_kernel_spmd(_NC, in_maps, core_ids=list(range(B)))
    else:
        res = run_bass_kernel_spmd(_NC, in_maps, core_ids=list(range(B)))
    LAST_EXEC_NS = res.exec_time_ns
    return np.stack([res.results[b]["out"] for b in range(B)], axis=0)


# revision 15
# speedup vs baseline: 1.1768x; 1.0177x over previous
"""Batch-parallel attention kernel for Trainium2 (8 NeuronCores).

Problem: out[b,j,d] = sum_i softmax_j(enc[b] @ dec[b].T)[i,j] * enc[b,i,d]
  enc/dec: [8, 2048, 512] fp32.  One batch per core (data parallel).

Per-core algorithm (batch b):
  S = enc @ dec.T        [2048, 2048]  (fp16 hi/lo 3-pass matmul, fp32-grade)
  A = softmax(S, axis=1) (two-half online softmax: local max/sum per
                          1024-wide half, combined into global stats)
  out = A.T @ enc        (fp16 matmul; exp(m_h-m)/L folded into enc rows,
                          one scaled enc variant per half)

Matmul layouts (out = lhsT.T @ rhs, contraction over partitions):
  MM1: lhsT = encT [d,i] chunks, rhs = decT [d,j] -> S[i,j] in PSUM.
       encT/decT produced by PE-transpose (identity matmul), split into
       fp16 hi + fp16 lo planes for the 3-pass product.
  MM2: lhsT = P[i,j] block (natural layout), rhs = enc_scaled[i,h,d] fp16.
"""

import os
import sys

sys.path.insert(0, "/opt/trn_rl_repo")

from contextlib import ExitStack

import numpy as np

import concourse.bass as bass  # noqa: F401
import concourse.bacc as bacc
import concourse.mybir as mybir
import concourse.tile as tile
from concourse.masks import make_identity
from concourse.bass_utils import run_bass_kernel_spmd

F32 = mybir.dt.float32
F16 = mybir.dt.float16
AX = mybir.AxisListType
ALU = mybir.AluOpType
ACTF = mybir.ActivationFunctionType

B, S_LEN, D = 8, 2048, 512
IB = S_LEN // 128   # 16 row blocks
KC = D // 128       # 4 contraction chunks
JT = S_LEN // 128   # 16 out row blocks
HALF = S_LEN // 2   # 1024

LAST_EXEC_NS = None


def _build(repeat=1, mm1_passes=3, skip_mm2=False):
    nc = bacc.Bacc()
    enc = nc.declare_dram_parameter("enc", [S_LEN, D], F32, isOutput=False)
    dec = nc.declare_dram_parameter("dec", [S_LEN, D], F32, isOutput=False)
    out = nc.declare_dram_parameter("out", [S_LEN, D], F32, isOutput=True)

    with ExitStack() as ctx:
        tc = ctx.enter_context(tile.TileContext(nc))
        if repeat > 1:
            ctx.enter_context(tc.For_i(0, repeat, 1))
        singles = ctx.enter_context(tc.tile_pool(name="singles", bufs=1))
        ld = ctx.enter_context(tc.tile_pool(name="ld", bufs=3))
        small = ctx.enter_context(tc.tile_pool(name="small", bufs=4))
        stage = ctx.enter_context(tc.tile_pool(name="stage", bufs=3))
        enc_ld = ctx.enter_context(tc.tile_pool(name="enc_ld", bufs=4))
        psum_s = ctx.enter_context(tc.tile_pool(name="psum_s", bufs=2, space="PSUM"))
        t_stack = ExitStack()
        psum_t = t_stack.enter_context(tc.tile_pool(name="psum_t", bufs=4, space="PSUM"))

        ident = singles.tile([128, 128], F32)
        make_identity(nc, ident)

        # resident tensors (per-partition bytes: 4*16K f16 = 64K + P 64K + 16K)
        encT_hi = singles.tile([128, KC, S_LEN], F16)   # [d, kc, i]
        encT_lo = singles.tile([128, KC, S_LEN], F16)
        decT_hi = singles.tile([128, KC, S_LEN], F16)   # [d, kc, j]
        decT_lo = singles.tile([128, KC, S_LEN], F16)
        enc_sc = singles.tile([128, IB, 2, D], F16)     # [i, ib, half, d]
        P = singles.tile([128, IB, S_LEN], F16)         # [i, ib, j]

        def transp4(src_sb, dst_hi, dst_lo, jsl):
            # transpose 4 [128,128] chunks of src_sb [128,512] into one PSUM
            # bank, then split to fp16 hi/lo planes with one wide op each
            pt = psum_t.tile([128, 512], F32, tag="pt", name="pt")
            for k in range(KC):
                nc.tensor.transpose(pt[:, k * 128:(k + 1) * 128],
                                    src_sb[:, k * 128:(k + 1) * 128], ident)
            ptv = pt.rearrange("p (k c) -> p k c", k=KC)
            nc.scalar.copy(out=dst_hi[:, :, jsl], in_=ptv)
            nc.vector.scalar_tensor_tensor(
                out=dst_lo[:, :, jsl], in0=dst_hi[:, :, jsl], scalar=-1.0,
                in1=ptv, op0=ALU.mult, op1=ALU.add)

        # spread DMA issue across the HWDGE issuing engines — the SP
        # sequencer alone serializes at ~0.57us per dma_start
        dma_engines = [nc.sync, nc.scalar]
        dma_rr = [0]

        def dma(out_ap, in_ap):
            eng = dma_engines[dma_rr[0] % len(dma_engines)]
            dma_rr[0] += 1
            eng.dma_start(out=out_ap, in_=in_ap)

        # ---- dec: load + transpose + split (jb 0..7 up front; 8..15 are
        # emitted inside ib=0 so MM1[ib0,h0] compute hides their DMA wait) --
        def dec_block(jb):
            dec_sb = ld.tile([128, D], F32, tag="dec_sb", name="dec_sb")
            dma(dec_sb, dec[jb * 128:(jb + 1) * 128, :])
            transp4(dec_sb, decT_hi, decT_lo, slice(jb * 128, (jb + 1) * 128))

        for jb in range(IB // 2):
            dec_block(jb)

        # ---- phase 1: per row block: enc transp, S halves, online softmax -
        # enc transposes run 2 iterations ahead so their ACT/DVE hi/lo splits
        # complete off the PE critical path; enc_sb tiles stay live until the
        # end-of-iteration scaling, so give them their own deep pool
        enc_sbs = {}

        def prefetch_enc(ib):
            if ib >= IB:
                return
            enc_sb = enc_ld.tile([128, D], F32, tag="enc_sb", name="enc_sb")
            dma(enc_sb, enc[ib * 128:(ib + 1) * 128, :])
            transp4(enc_sb, encT_hi, encT_lo, slice(ib * 128, (ib + 1) * 128))
            enc_sbs[ib] = enc_sb

        prefetch_enc(0)
        prefetch_enc(1)
        for ib in range(IB):
            isl = slice(ib * 128, (ib + 1) * 128)
            enc_sb = enc_sbs.pop(ib)

            passes = [(encT_hi, decT_hi), (encT_hi, decT_lo),
                      (encT_lo, decT_hi)][:mm1_passes]
            npass = len(passes)
            nm = [None, None]
            lp = [None, None]
            for h in range(2):
                if ib == 0 and h == 1:
                    # second dec half: its DMA+transposes hide under the
                    # h=0 matmul burst that was just emitted
                    for jb in range(IB // 2, IB):
                        dec_block(jb)
                Sh = psum_s.tile([128, HALF], F32, tag="S", name="S")
                for pi, (eT, dT) in enumerate(passes):
                    for k in range(KC):
                        for n in range(2):
                            nc.tensor.matmul(
                                Sh[:, n * 512:(n + 1) * 512],
                                lhsT=eT[:, k, isl],
                                rhs=dT[:, k, h * HALF + n * 512: h * HALF + (n + 1) * 512],
                                start=(pi == 0 and k == 0),
                                stop=(pi == npass - 1 and k == KC - 1))
                nm[h] = small.tile([128, 1], F32, tag=f"nm{h}", name=f"nm{h}")
                nc.vector.tensor_reduce(out=nm[h], in_=Sh, axis=AX.X,
                                        op=ALU.max, negate=True)
                lp[h] = small.tile([128, 1], F32, tag=f"lp{h}", name=f"lp{h}")
                nc.scalar.activation(out=P[:, ib, h * HALF:(h + 1) * HALF],
                                     in_=Sh, func=ACTF.Exp, bias=nm[h],
                                     scale=1.0, accum_out=lp[h])

            prefetch_enc(ib + 2)

            # combine halves: m = max(m0, m1); f_h = exp(m_h - m);
            # L = L0 f0 + L1 f1; r_h = f_h / L; enc_sc_h = enc * r_h
            negm = small.tile([128, 1], F32, tag="negm", name="negm")
            nc.vector.tensor_tensor(out=negm, in0=nm[0], in1=nm[1], op=ALU.min)
            f = [None, None]
            for h in range(2):
                df = small.tile([128, 1], F32, tag=f"df{h}", name=f"df{h}")
                nc.vector.tensor_tensor(out=df, in0=negm, in1=nm[h], op=ALU.subtract)
                f[h] = small.tile([128, 1], F32, tag=f"f{h}", name=f"f{h}")
                nc.scalar.activation(out=f[h], in_=df, func=ACTF.Exp)
            l1f1 = small.tile([128, 1], F32, tag="l1f1", name="l1f1")
            nc.vector.tensor_tensor(out=l1f1, in0=lp[1], in1=f[1], op=ALU.mult)
            L = small.tile([128, 1], F32, tag="L", name="L")
            nc.vector.scalar_tensor_tensor(out=L, in0=lp[0], scalar=f[0],
                                           in1=l1f1, op0=ALU.mult, op1=ALU.add)
            r = small.tile([128, 1], F32, tag="r", name="r")
            nc.vector.reciprocal(out=r, in_=L)
            for h in range(2):
                rh = small.tile([128, 1], F32, tag=f"rh{h}", name=f"rh{h}")
                nc.vector.tensor_tensor(out=rh, in0=r, in1=f[h], op=ALU.mult)
                nc.gpsimd.tensor_scalar(out=enc_sc[:, ib, h, :], in0=enc_sb,
                                        scalar1=rh, scalar2=None, op0=ALU.mult)

        # ---- phase 2: out = A.T @ enc_scaled ------------------------------
        t_stack.close()
        psum_o = ctx.enter_context(tc.tile_pool(name="psum_o", bufs=2, space="PSUM"))
        for jt in range(JT if not skip_mm2 else 1):
            po = psum_o.tile([128, D], F32, tag="po", name="po")
            h = jt // (JT // 2)
            for ib in range(IB):
                nc.tensor.matmul(po,
                                 lhsT=P[:, ib, jt * 128:(jt + 1) * 128],
                                 rhs=enc_sc[:, ib, h, :],
                                 start=(ib == 0), stop=(ib == IB - 1))
            st = stage.tile([128, D], F32, tag="st", name="st")
            nc.scalar.copy(out=st, in_=po)
            dma(out[jt * 128:(jt + 1) * 128, :], st)

    nc.compile()
    return nc


_NC = None


def kernel(enc_outputs, dec_outputs):
    global _NC, LAST_EXEC_NS
    enc_outputs = np.ascontiguousarray(np.asarray(enc_outputs, dtype=np.float32))
    dec_outputs = np.ascontiguousarray(np.asarray(dec_outputs, dtype=np.float32))
    assert enc_outputs.shape == (B, S_LEN, D), enc_outputs.shape
    assert dec_outputs.shape == (B, S_LEN, D), dec_outputs.shape

    if _NC is None:
        _NC = _build()

    in_maps = [{"enc": enc_outputs[b], "dec": dec_outputs[b]} for b in range(B)]
    trace = bool(int(os.environ.get("BASS_ATTN_TRACE", "0")))
    if trace:
        try:
            res = run_bass_kernel_spmd(_NC, in_maps, core_ids=list(range(B)), trace=True)
        except Exception:
            res = run_bass_kernel_spmd(_NC, in_maps, core_ids=list(range(B)))
    else:
        res = run_bass_kernel_spmd(_NC, in_maps, core_ids=list(range(B)))
    LAST_EXEC_NS = res.exec_time_ns
    return np.stack([res.results[b]["out"] for b in range(B)], axis=0)
